# revision 23
# baseline (speedup 1.0000x reference)
"""GPT-3 style multi-head attention on Trainium2, 8-core SPMD Bass kernel.

Problem shapes: B=2, S=4096, D=768, H=12, depth=64 (fp32).

Sharding (hardcoded): core c in 0..7 -> batch b = c//4, head group g = c%4
(3 heads per core).  ScalarE (exp over S x S logits per head) is the
bottleneck engine; the schedule keeps it saturated:
  - x / weights are cast to bf16 (gpsimd casting DMA), transposed on PE
    (1 cyc/row, 1-bank psum tiles), projected with bf16 matmuls,
  - prologue projects K, Q(blocks 0-1) and V while running q-blocks 0-1,
    heads 0&1 attention kc-major one seq-chunk behind the K projection,
  - QK^T for heads 0&1 issued as a row-tiled pair (K=64 each, the two
    PE array halves run concurrently), exp on ScalarE covers both heads
    in one instruction, AV appends a ones column for the softmax
    denominator; head 2 self-pairs over (even, odd) key chunks using
    partition-duplicated qT/kT built free via duplicated weight columns,
  - main loop: per q-block Q projection, attention, and the previous
    block's output projection all under the exp shadow.
Host sums the 4 partials per batch and adds the output bias bo.
"""

import numpy as np

import concourse.bacc as bacc
import concourse.mybir as mybir
import concourse.tile as tile
from concourse import bass_utils
from concourse.masks import make_identity

B, S, D, H = 2, 4096, 768, 12
DEPTH = 64
HPC = 3                 # heads per core
GW = HPC * DEPTH        # 192: head-group width
N_CORES = 8
SCALE = 1.0 / float(np.sqrt(DEPTH))

F32 = mybir.dt.float32
F32R = mybir.dt.float32r
BF16 = mybir.dt.bfloat16
AF = mybir.ActivationFunctionType

P = 128
FCH = D // P            # 6 feature chunks
NSP = S // (2 * P)      # 16 seq pairs (256 rows each)
NKC = S // P            # 32 key chunks
QB = 512                # q block width
NQB = S // QB           # 8

# set by test.py to get a traced run
TRACE = False
LAST_RESULTS = None


def _emit(nc, tc, ctx, tensors, repeat=1, phases="ABC"):
    setup = _emit_setup(nc, tc, ctx, tensors)
    for _ in range(repeat):
        _emit_compute(nc, tc, tensors, setup, phases=phases)


def _emit_setup(nc, tc, ctx, tensors):
    XQ, XK, XV, WQ, WK, WV, WO, BQ, BK, BV, OUT = tensors

    const = ctx.enter_context(tc.tile_pool(name="const", bufs=1))

    ident_f = const.tile([P, P], F32)
    make_identity(nc, ident_f[:])
    ident_r = const.tile([P, P], F32R)
    nc.vector.tensor_copy(ident_r[:], ident_f[:])
    ident = ident_r[:]

    # biases as per-partition columns; head-2 slices duplicated into both
    # partition halves
    bq01 = const.tile([P, 1], F32)
    nc.sync.dma_start(bq01[:], BQ[0:P, :])
    bq2d = const.tile([P, 1], F32)
    nc.sync.dma_start(bq2d[0:DEPTH, :], BQ[P:GW, :])
    nc.sync.dma_start(bq2d[DEPTH:P, :], BQ[P:GW, :])
    bk01 = const.tile([P, 1], F32)
    nc.sync.dma_start(bk01[:], BK[0:P, :])
    bk2d = const.tile([P, 1], F32)
    nc.sync.dma_start(bk2d[0:DEPTH, :], BK[P:GW, :])
    nc.sync.dma_start(bk2d[DEPTH:P, :], BK[P:GW, :])
    # bv broadcast across partitions for the v-natural layout
    bvrow = const.tile([1, GW], F32)
    nc.sync.dma_start(bvrow[:], BV[:, :])
    bvb = const.tile([P, GW], F32)
    nc.gpsimd.partition_broadcast(bvb[:], bvrow[:])

    # weights: straight DMA into f32r tiles (same bits as f32)
    wq01 = const.tile([P, FCH, P], F32R)
    wq2d = const.tile([P, FCH, P], F32R)
    wk01 = const.tile([P, FCH, P], F32R)
    wk2d = const.tile([P, FCH, P], F32R)
    wre_q = WQ.rearrange("(c p) n -> p c n", p=P)
    wre_k = WK.rearrange("(c p) n -> p c n", p=P)
    nc.sync.dma_start(wq01[:], wre_q[:, :, 0:P])
    nc.sync.dma_start(wq2d[:, :, 0:DEPTH], wre_q[:, :, P:GW])
    nc.sync.dma_start(wq2d[:, :, DEPTH:P], wre_q[:, :, P:GW])
    nc.sync.dma_start(wk01[:], wre_k[:, :, 0:P])
    nc.sync.dma_start(wk2d[:, :, 0:DEPTH], wre_k[:, :, P:GW])
    nc.sync.dma_start(wk2d[:, :, DEPTH:P], wre_k[:, :, P:GW])
    wv = const.tile([P, FCH, 2 * GW], F32R)  # duplicated so N>=256
    wre_v = WV.rearrange("(c p) n -> p c n", p=P)
    nc.sync.dma_start(wv[:, :, 0:GW], wre_v)
    nc.sync.dma_start(wv[:, :, GW:2 * GW], wre_v)
    wo0 = const.tile([P, D], F32R)
    nc.sync.dma_start(wo0[:], WO[0:P, :])
    wo1 = const.tile([DEPTH, D], F32R)
    nc.sync.dma_start(wo1[:], WO[P:GW, :])

    # persistent attention operands
    qT01 = const.tile([P, S], F32R)
    qT2f = const.tile([P, S], F32R)   # head 2, duplicated partition halves
    kT01 = const.tile([P, S], F32R)
    kT2f = const.tile([P, S], F32R)
    vht = const.tile([P, NKC, HPC, DEPTH + 1], F32R)
    ones_t = const.tile([P, NKC], F32)
    nc.gpsimd.memset(ones_t[:], 1.0)
    for h in range(HPC):
        nc.vector.tensor_copy(vht[:, :, h, DEPTH], ones_t[:])
    hout01 = const.tile([P, S], F32R)
    hout2 = const.tile([DEPTH, S], F32R)

    return dict(
        ident=ident, bq01=bq01, bq2d=bq2d, bk01=bk01, bk2d=bk2d, bvb=bvb,
        wq01=wq01, wq2d=wq2d, wk01=wk01, wk2d=wk2d, wv=wv, wo0=wo0, wo1=wo1,
        qT01=qT01, qT2f=qT2f, kT01=kT01, kT2f=kT2f, vht=vht,
        hout01=hout01, hout2=hout2,
    )


def _emit_compute(nc, tc, tensors, st, phases="ABC"):
    """Prologue projects K, Q(blocks 0-1) and V while running q-blocks 0-1
    heads-0/1 attention one seq-chunk behind; the main loop runs the rest
    ACT-bound with Q projection and output projection under the exp
    shadow."""
    XQ, XK, XV, WQ, WK, WV, WO, BQ, BK, BV, OUT = tensors
    ident, bvb = st["ident"], st["bvb"]
    bq01, bq2d = st["bq01"], st["bq2d"]
    bk01, bk2d = st["bk01"], st["bk2d"]
    wq01, wq2d = st["wq01"], st["wq2d"]
    wk01, wk2d, wv = st["wk01"], st["wk2d"], st["wv"]
    wo0, wo1 = st["wo0"], st["wo1"]
    qT01, qT2f, kT01, kT2f = st["qT01"], st["qT2f"], st["kT01"], st["kT2f"]
    vht, hout01, hout2 = st["vht"], st["hout01"], st["hout2"]

    xre_q = XQ.rearrange("(sp a p) d -> sp p a d", a=2, p=P)
    xre_k = XK.rearrange("(sp a p) d -> sp p a d", a=2, p=P)
    xre_v = XV.rearrange("(sp a p) d -> sp p a d", a=2, p=P)

    with (
        tc.tile_pool(name="xnat", bufs=3) as xnat_pool,
        tc.tile_pool(name="xts", bufs=2) as xts_pool,
        tc.tile_pool(name="ex", bufs=3) as ex_pool,
        tc.tile_pool(name="nrm", bufs=2) as nrm_pool,
        tc.tile_pool(name="outt", bufs=2) as out_pool,
    ):
        # ---------- shared helpers (psum pools passed in) ----------
        def load_xn(xre, sp):
            xn = xnat_pool.tile([P, 2, D], F32R, tag="xn", name="xn")
            nc.sync.dma_start(xn[:], xre[sp, :, :, :])
            return xn

        def transpose_sp(xn, psum_pool, tag="pr"):
            xt = xts_pool.tile([P, FCH, 2 * P], F32R, tag="xt", name="xt")
            for a in range(2):
                tp = psum_pool.tile([P, FCH * P], F32R, tag=tag, name="tp")
                for f in range(FCH):
                    nc.tensor.transpose(
                        tp[:, f * P:(f + 1) * P],
                        xn[:, a, f * P:(f + 1) * P], ident)
                nc.vector.tensor_copy(xt[:, :, a * P:(a + 1) * P], tp[:])
            return xt

        def transpose_sp_split(xn, psum_pool, tag="aux"):
            # 1-bank psum tiles for the shared main-loop aux pool
            xt = xts_pool.tile([P, FCH, 2 * P], F32R, tag="xt", name="xt")
            for a in range(2):
                t0 = psum_pool.tile([P, 4 * P], F32R, tag=tag, name="t0")
                for f in range(4):
                    nc.tensor.transpose(
                        t0[:, f * P:(f + 1) * P],
                        xn[:, a, f * P:(f + 1) * P], ident)
                nc.vector.tensor_copy(xt[:, 0:4, a * P:(a + 1) * P], t0[:])
                t1 = psum_pool.tile([P, 2 * P], F32R, tag=tag, name="t1")
                for f in range(2):
                    nc.tensor.transpose(
                        t1[:, f * P:(f + 1) * P],
                        xn[:, a, (4 + f) * P:(5 + f) * P], ident)
                nc.vector.tensor_copy(xt[:, 4:6, a * P:(a + 1) * P], t1[:])
            return xt

        def proj_v(sp, xt, psum_pool, tag="pr"):
            for a in range(2):
                pv = psum_pool.tile([P, 2 * GW], F32, tag=tag, name="pv")
                for f in range(FCH):
                    nc.tensor.matmul(
                        pv[:], xt[:, f, a * P:(a + 1) * P], wv[:, f, :],
                        start=(f == 0), stop=(f == FCH - 1),
                    )
                s2 = sp * 2 + a
                nc.vector.tensor_add(
                    vht[:, s2, :, 0:DEPTH],
                    pv[:, 0:GW].rearrange("p (h d) -> p h d", h=HPC),
                    bvb[:].rearrange("p (h d) -> p h d", h=HPC),
                )

        def proj_qk(sp, xt, psum_pool, w01, w2d, b01, b2d, d01, d2f,
                    tag="pr"):
            sl = slice(sp * 2 * P, (sp + 1) * 2 * P)
            p01 = psum_pool.tile([P, 2 * P], F32, tag=tag, name="p01")
            for f in range(FCH):
                nc.tensor.matmul(
                    p01[:], w01[:, f, :], xt[:, f, :],
                    start=(f == 0), stop=(f == FCH - 1),
                )
            nc.vector.tensor_scalar_add(d01[:, sl], p01[:], b01[:])
            p2d = psum_pool.tile([P, 2 * P], F32, tag=tag, name="p2d")
            for f in range(FCH):
                nc.tensor.matmul(
                    p2d[:], w2d[:, f, :], xt[:, f, :],
                    start=(f == 0), stop=(f == FCH - 1),
                )
            nc.vector.tensor_scalar_add(d2f[:, sl], p2d[:], b2d[:])

        def normalize(outp, dst):
            rc = nrm_pool.tile([1, QB], F32, tag="rc", name="rc")
            nc.vector.reciprocal(rc[:], outp[DEPTH:DEPTH + 1, :])
            bc = nrm_pool.tile([DEPTH, QB], F32, tag="bc", name="bc")
            nc.gpsimd.partition_broadcast(bc[:], rc[:])
            nc.vector.tensor_mul(dst, outp[0:DEPTH, :], bc[:])

        def qk01(kc, qsl, lg_pool):
            lg = lg_pool.tile([P, 2, QB], F32, tag="lg", name="lg")
            nc.tensor.matmul(
                lg[:, 0, :], kT01[0:DEPTH, kc * P:(kc + 1) * P],
                qT01[0:DEPTH, qsl], start=True, stop=True)
            nc.tensor.matmul(
                lg[:, 1, :], kT01[DEPTH:P, kc * P:(kc + 1) * P],
                qT01[DEPTH:P, qsl], start=True, stop=True)
            return lg

        def av01(kc, lg, outp0, outp1):
            ext = ex_pool.tile([P, 2, QB], F32R, tag="ex", name="ex")
            nc.scalar.activation(ext[:], lg[:], AF.Exp, scale=SCALE)
            nc.tensor.matmul(
                outp0[:], vht[:, kc, 0, :], ext[:, 0, :],
                start=(kc == 0), stop=(kc == NKC - 1))
            nc.tensor.matmul(
                outp1[:], vht[:, kc, 1, :], ext[:, 1, :],
                start=(kc == 0), stop=(kc == NKC - 1))

        def qk2(j, qsl, lg_pool):
            lg = lg_pool.tile([P, 2, QB], F32, tag="lg", name="lg")
            nc.tensor.matmul(
                lg[:, 0, :], kT2f[0:DEPTH, (2 * j) * P:(2 * j + 1) * P],
                qT2f[0:DEPTH, qsl], start=True, stop=True)
            nc.tensor.matmul(
                lg[:, 1, :],
                kT2f[DEPTH:P, (2 * j + 1) * P:(2 * j + 2) * P],
                qT2f[DEPTH:P, qsl], start=True, stop=True)
            return lg

        def av2(j, lg, outp2):
            ext = ex_pool.tile([P, 2, QB], F32R, tag="ex", name="ex")
            nc.scalar.activation(ext[:], lg[:], AF.Exp, scale=SCALE)
            nc.tensor.matmul(
                outp2[:], vht[:, 2 * j, 2, :], ext[:, 0, :],
                start=(j == 0), stop=False)
            nc.tensor.matmul(
                outp2[:], vht[:, 2 * j + 1, 2, :], ext[:, 1, :],
                start=False, stop=(j == NKC // 2 - 1))

        # ---------- phase 1: project V then K (pipelined) ----------
        with (
            tc.tile_pool(name="tps", bufs=2, space="PSUM") as tps_pool,
            tc.tile_pool(name="pps", bufs=4, space="PSUM") as pps_pool,
        ):
            def vk_unit(kind, sp):
                xn = load_xn(xre_v if kind == "v" else xre_k, sp)
                return transpose_sp(xn, tps_pool, tag="tp")

            def vk_proj(kind, sp, xt):
                if kind == "v":
                    proj_v(sp, xt, pps_pool, tag="pp")
                else:
                    proj_qk(sp, xt, pps_pool, wk01, wk2d, bk01, bk2d,
                            kT01, kT2f, tag="pp")

            steps = [("v", sp) for sp in range(NSP)] +                     [("k", sp) for sp in range(NSP)]
            prev = None
            for kind, sp in steps:
                xt = vk_unit(kind, sp)
                if prev is not None:
                    vk_proj(prev[0], prev[1], prev[2])
                prev = (kind, sp, xt)
            vk_proj(prev[0], prev[1], prev[2])

        # ---------- phase 2: per q-block attention + output proj ----------
        with (
            tc.tile_pool(name="lg", bufs=2, space="PSUM") as lg_pool,
            tc.tile_pool(name="op", bufs=2, space="PSUM") as op_pool,
            tc.tile_pool(name="aux", bufs=2, space="PSUM") as aux_pool,
        ):
            def proj_q_main(sp):
                xn = load_xn(xre_q, sp)
                xt = transpose_sp_split(xn, aux_pool, tag="aux")
                proj_qk(sp, xt, aux_pool, wq01, wq2d, bq01, bq2d,
                        qT01, qT2f, tag="aux")

            def pass_h01(qb):
                qsl = slice(qb * QB, (qb + 1) * QB)
                o0 = op_pool.tile([DEPTH + 1, QB], F32, tag="outp",
                                  name="outp0")
                o1 = op_pool.tile([DEPTH + 1, QB], F32, tag="outp",
                                  name="outp1")
                prev = qk01(0, qsl, lg_pool)
                for kc in range(1, NKC):
                    cur = qk01(kc, qsl, lg_pool)
                    av01(kc - 1, prev, o0, o1)
                    prev = cur
                av01(NKC - 1, prev, o0, o1)
                normalize(o0, hout01[0:DEPTH, qsl])
                normalize(o1, hout01[DEPTH:P, qsl])

            def pass_h2(qb):
                qsl = slice(qb * QB, (qb + 1) * QB)
                o2 = op_pool.tile([DEPTH + 1, QB], F32, tag="outp",
                                  name="outp2")
                prev = qk2(0, qsl, lg_pool)
                for j in range(1, NKC // 2):
                    cur = qk2(j, qsl, lg_pool)
                    av2(j - 1, prev, o2)
                    prev = cur
                av2(NKC // 2 - 1, prev, o2)
                normalize(o2, hout2[:, qsl])

            def emit_c(qb):
                for m in range(4 * qb, 4 * qb + 4):
                    msl = slice(m * P, (m + 1) * P)
                    pa = aux_pool.tile([P, 512], F32, tag="aux", name="pa")
                    nc.tensor.matmul(pa[:], hout01[:, msl], wo0[:, 0:512],
                                     start=True, stop=False)
                    nc.tensor.matmul(pa[:], hout2[:, msl], wo1[:, 0:512],
                                     start=False, stop=True)
                    pb = aux_pool.tile([P, 256], F32, tag="aux", name="pb")
                    nc.tensor.matmul(pb[:], hout01[:, msl], wo0[:, 512:D],
                                     start=True, stop=False)
                    nc.tensor.matmul(pb[:], hout2[:, msl], wo1[:, 512:D],
                                     start=False, stop=True)
                    ot = out_pool.tile([P, D], F32, tag="ot", name="ot")
                    nc.vector.tensor_copy(ot[:, 0:512], pa[:])
                    nc.vector.tensor_copy(ot[:, 512:D], pb[:])
                    nc.sync.dma_start(OUT[msl, :], ot[:])

            for qb in range(NQB):
                proj_q_main(2 * qb)
                proj_q_main(2 * qb + 1)
                pass_h01(qb)
                if qb > 0:
                    emit_c(qb - 1)
                pass_h2(qb)
            emit_c(NQB - 1)


_NC = None


def build_nc(repeat=1, phases="ABC"):
    nc = bacc.Bacc("TRN2", target_bir_lowering=False, debug=False)
    XQ = nc.dram_tensor("xq", [S, D], F32R, kind="ExternalInput").ap()
    XK = nc.dram_tensor("xk", [S, D], F32R, kind="ExternalInput").ap()
    XV = nc.dram_tensor("xv", [S, D], F32R, kind="ExternalInput").ap()
    WQ = nc.dram_tensor("wq", [D, GW], F32R, kind="ExternalInput").ap()
    WK = nc.dram_tensor("wk", [D, GW], F32R, kind="ExternalInput").ap()
    WV = nc.dram_tensor("wv", [D, GW], F32R, kind="ExternalInput").ap()
    WO = nc.dram_tensor("wo", [GW, D], F32R, kind="ExternalInput").ap()
    BQ = nc.dram_tensor("bq", [GW, 1], F32, kind="ExternalInput").ap()
    BK = nc.dram_tensor("bk", [GW, 1], F32, kind="ExternalInput").ap()
    BV = nc.dram_tensor("bv", [1, GW], F32, kind="ExternalInput").ap()
    OUT = nc.dram_tensor("out", [S, D], F32, kind="ExternalOutput").ap()
    tensors = (XQ, XK, XV, WQ, WK, WV, WO, BQ, BK, BV, OUT)
    from contextlib import ExitStack
    with tile.TileContext(nc) as tc:
        with ExitStack() as ctx:
            _emit(nc, tc, ctx, tensors, repeat=repeat, phases=phases)
    nc.compile()
    return nc


def _get_nc():
    global _NC
    if _NC is None:
        _NC = build_nc()
    return _NC


def kernel(**inputs):
    global LAST_RESULTS
    q = np.ascontiguousarray(np.asarray(inputs["q"], dtype=np.float32))
    k = np.ascontiguousarray(np.asarray(inputs["k"], dtype=np.float32))
    v = np.ascontiguousarray(np.asarray(inputs["v"], dtype=np.float32))
    Wq = np.asarray(inputs["Wq"], dtype=np.float32)
    Wk = np.asarray(inputs["Wk"], dtype=np.float32)
    Wv = np.asarray(inputs["Wv"], dtype=np.float32)
    Wo = np.asarray(inputs["Wo"], dtype=np.float32)
    bq = np.asarray(inputs["bq"], dtype=np.float32)
    bk = np.asarray(inputs["bk"], dtype=np.float32)
    bv = np.asarray(inputs["bv"], dtype=np.float32)
    bo = np.asarray(inputs["bo"], dtype=np.float32)
    # mask is all zeros by problem spec; ignored.

    nc = _get_nc()
    in_maps = []
    for c in range(N_CORES):
        b, g = c // 4, c % 4
        sl = slice(g * GW, (g + 1) * GW)
        in_maps.append({
            "xq": q[b], "xk": k[b], "xv": v[b],
            "wq": np.ascontiguousarray(Wq[:, sl]),
            "wk": np.ascontiguousarray(Wk[:, sl]),
            "wv": np.ascontiguousarray(Wv[:, sl]),
            "wo": np.ascontiguousarray(Wo[sl, :]),
            "bq": np.ascontiguousarray(bq[sl].reshape(GW, 1)),
            "bk": np.ascontiguousarray(bk[sl].reshape(GW, 1)),
            "bv": np.ascontiguousarray(bv[sl].reshape(1, GW)),
        })
    kwargs = {}
    if TRACE:
        kwargs = dict(trace=True)
    res = bass_utils.run_bass_kernel_spmd(nc, in_maps, list(range(N_CORES)),
                                          **kwargs)
    LAST_RESULTS = res
    out = np.zeros((B, S, D), dtype=np.float32)
    for c in range(N_CORES):
        out[c // 4] += res.results[c]["out"]
    out += bo
    return out


# revision 24
# speedup vs baseline: 1.6656x; 1.6656x over previous
"""GPT-3 style multi-head attention on Trainium2, 8-core SPMD Bass kernel.

Problem shapes: B=2, S=4096, D=768, H=12, depth=64 (fp32).

Sharding (hardcoded): core c in 0..7 -> batch b = c//4, head group g = c%4
(3 heads per core).  ScalarE (exp over S x S logits per head) is the
bottleneck engine; the schedule keeps it saturated:
  - x / weights are cast to bf16 (gpsimd casting DMA), transposed on PE
    (1 cyc/row, 1-bank psum tiles), projected with bf16 matmuls,
  - prologue projects K, Q(blocks 0-1) and V while running q-blocks 0-1,
    heads 0&1 attention kc-major one seq-chunk behind the K projection,
  - QK^T for heads 0&1 issued as a row-tiled pair (K=64 each, the two
    PE array halves run concurrently), exp on ScalarE covers both heads
    in one instruction, AV appends a ones column for the softmax
    denominator; head 2 self-pairs over (even, odd) key chunks using
    partition-duplicated qT/kT built free via duplicated weight columns,
  - main loop: per q-block Q projection, attention, and the previous
    block's output projection all under the exp shadow.
Host sums the 4 partials per batch and adds the output bias bo.
"""

import numpy as np

import concourse.bacc as bacc
import concourse.mybir as mybir
import concourse.tile as tile
from concourse import bass_utils
from concourse.masks import make_identity

B, S, D, H = 2, 4096, 768, 12
DEPTH = 64
HPC = 3                 # heads per core
GW = HPC * DEPTH        # 192: head-group width
N_CORES = 8
SCALE = 1.0 / float(np.sqrt(DEPTH))

F32 = mybir.dt.float32
F32R = mybir.dt.float32r
BF16 = mybir.dt.bfloat16
AF = mybir.ActivationFunctionType

P = 128
FCH = D // P            # 6 feature chunks
NSP = S // (2 * P)      # 16 seq pairs (256 rows each)
NKC = S // P            # 32 key chunks
QB = 512                # q block width
NQB = S // QB           # 8

# set by test.py to get a traced run
TRACE = False
LAST_RESULTS = None


def _emit(nc, tc, ctx, tensors, repeat=1, phases="ABC"):
    setup = _emit_setup(nc, tc, ctx, tensors)
    for _ in range(repeat):
        _emit_compute(nc, tc, tensors, setup, phases=phases)


def _emit_setup(nc, tc, ctx, tensors):
    XQ, XK, XV, WQ, WK, WV, WO, BQ, BK, BV, OUT = tensors

    const = ctx.enter_context(tc.tile_pool(name="const", bufs=1))

    ident_f = const.tile([P, P], F32)
    make_identity(nc, ident_f[:])
    ident_b = const.tile([P, P], BF16)
    nc.vector.tensor_copy(ident_b[:], ident_f[:])
    ident = ident_b[:]

    # biases as per-partition columns; head-2 slices duplicated into both
    # partition halves
    bq01 = const.tile([P, 1], F32)
    nc.sync.dma_start(bq01[:], BQ[0:P, :])
    bq2d = const.tile([P, 1], F32)
    nc.sync.dma_start(bq2d[0:DEPTH, :], BQ[P:GW, :])
    nc.sync.dma_start(bq2d[DEPTH:P, :], BQ[P:GW, :])
    bk01 = const.tile([P, 1], F32)
    nc.sync.dma_start(bk01[:], BK[0:P, :])
    bk2d = const.tile([P, 1], F32)
    nc.sync.dma_start(bk2d[0:DEPTH, :], BK[P:GW, :])
    nc.sync.dma_start(bk2d[DEPTH:P, :], BK[P:GW, :])
    # bv broadcast across partitions for the v-natural layout
    bvrow = const.tile([1, GW], F32)
    nc.sync.dma_start(bvrow[:], BV[:, :])
    bvb = const.tile([P, GW], F32)
    nc.gpsimd.partition_broadcast(bvb[:], bvrow[:])

    # weights: f32 staging DMAs on the SP queue, DVE-converted to bf16
    wq01 = const.tile([P, FCH, P], BF16)
    wq2d = const.tile([P, FCH, P], BF16)
    wk01 = const.tile([P, FCH, P], BF16)
    wk2d = const.tile([P, FCH, P], BF16)
    wv = const.tile([P, FCH, 2 * GW], BF16)  # duplicated so N>=256
    wo0 = const.tile([P, D], F32R)
    wo1 = const.tile([DEPTH, D], F32R)
    wre_q = WQ.rearrange("(c p) n -> p c n", p=P)
    wre_k = WK.rearrange("(c p) n -> p c n", p=P)
    wre_v = WV.rearrange("(c p) n -> p c n", p=P)
    with tc.tile_pool(name="wstage", bufs=2) as wstage:
        wq_s = wstage.tile([P, FCH, GW], F32, tag="ws", name="ws")
        nc.sync.dma_start(wq_s[:], wre_q)
        nc.vector.tensor_copy(wq01[:], wq_s[:, :, 0:P])
        nc.vector.tensor_copy(wq2d[:, :, 0:DEPTH], wq_s[:, :, P:GW])
        nc.vector.tensor_copy(wq2d[:, :, DEPTH:P], wq_s[:, :, P:GW])
        wk_s = wstage.tile([P, FCH, GW], F32, tag="ws", name="ws")
        nc.sync.dma_start(wk_s[:], wre_k)
        nc.vector.tensor_copy(wk01[:], wk_s[:, :, 0:P])
        nc.vector.tensor_copy(wk2d[:, :, 0:DEPTH], wk_s[:, :, P:GW])
        nc.vector.tensor_copy(wk2d[:, :, DEPTH:P], wk_s[:, :, P:GW])
        wv_s = wstage.tile([P, FCH, GW], F32, tag="ws", name="ws")
        nc.sync.dma_start(wv_s[:], wre_v)
        nc.vector.tensor_copy(wv[:, :, 0:GW], wv_s[:])
        nc.vector.tensor_copy(wv[:, :, GW:2 * GW], wv_s[:])
        wo_s = wstage.tile([P, D], F32, tag="ws", name="ws")
        nc.sync.dma_start(wo_s[:], WO[0:P, :])
        nc.vector.tensor_copy(wo0[:], wo_s[:])
        wo1_s = wstage.tile([DEPTH, D], F32, tag="ws", name="ws")
        nc.sync.dma_start(wo1_s[:], WO[P:GW, :])
        nc.vector.tensor_copy(wo1[:], wo1_s[:])

    # persistent attention operands
    qT01 = const.tile([P, S], F32R)
    qT2f = const.tile([P, S], F32R)   # head 2, duplicated partition halves
    kT01 = const.tile([P, S], F32R)
    kT2f = const.tile([P, S], F32R)
    vht = const.tile([P, NKC, HPC, DEPTH + 1], F32R)
    ones_t = const.tile([P, NKC], F32)
    nc.gpsimd.memset(ones_t[:], 1.0)
    for h in range(HPC):
        nc.vector.tensor_copy(vht[:, :, h, DEPTH], ones_t[:])
    hout01 = const.tile([P, S], F32R)
    hout2 = const.tile([DEPTH, S], F32R)

    return dict(
        ident=ident, bq01=bq01, bq2d=bq2d, bk01=bk01, bk2d=bk2d, bvb=bvb,
        wq01=wq01, wq2d=wq2d, wk01=wk01, wk2d=wk2d, wv=wv, wo0=wo0, wo1=wo1,
        qT01=qT01, qT2f=qT2f, kT01=kT01, kT2f=kT2f, vht=vht,
        hout01=hout01, hout2=hout2,
    )


def _emit_compute(nc, tc, tensors, st, phases="ABC"):
    """Prologue projects K, Q(blocks 0-1) and V while running q-blocks 0-1
    heads-0/1 attention one seq-chunk behind; the main loop runs the rest
    ACT-bound with Q projection and output projection under the exp
    shadow."""
    XQ, XK, XV, WQ, WK, WV, WO, BQ, BK, BV, OUT = tensors
    ident, bvb = st["ident"], st["bvb"]
    bq01, bq2d = st["bq01"], st["bq2d"]
    bk01, bk2d = st["bk01"], st["bk2d"]
    wq01, wq2d = st["wq01"], st["wq2d"]
    wk01, wk2d, wv = st["wk01"], st["wk2d"], st["wv"]
    wo0, wo1 = st["wo0"], st["wo1"]
    qT01, qT2f, kT01, kT2f = st["qT01"], st["qT2f"], st["kT01"], st["kT2f"]
    vht, hout01, hout2 = st["vht"], st["hout01"], st["hout2"]

    xre_q = XQ.rearrange("(sp a p) d -> sp p a d", a=2, p=P)
    xre_k = XK.rearrange("(sp a p) d -> sp p a d", a=2, p=P)
    xre_v = XV.rearrange("(sp a p) d -> sp p a d", a=2, p=P)

    with (
        tc.tile_pool(name="xnf", bufs=2) as xnf_pool,
        tc.tile_pool(name="xnat", bufs=2) as xnat_pool,
        tc.tile_pool(name="xts", bufs=2) as xts_pool,
        tc.tile_pool(name="ex", bufs=3) as ex_pool,
        tc.tile_pool(name="nrm", bufs=4) as nrm_pool,
        tc.tile_pool(name="outt", bufs=3) as out_pool,
    ):
        # ---------- shared helpers (psum pools passed in) ----------
        def load_xn(xre, sp):
            # f32 load on the SP HWDGE queue, DVE-converted to bf16
            xf = xnf_pool.tile([P, 2, D], F32, tag="xf", name="xf")
            nc.sync.dma_start(xf[:], xre[sp, :, :, :])
            xn = xnat_pool.tile([P, 2, D], BF16, tag="xn", name="xn")
            nc.vector.tensor_copy(xn[:], xf[:])
            return xn

        def transpose_sp(xn, psum_pool, tag="pr"):
            xt = xts_pool.tile([P, FCH, 2 * P], BF16, tag="xt", name="xt")
            for a in range(2):
                tp = psum_pool.tile([P, FCH * P], BF16, tag=tag, name="tp")
                for f in range(FCH):
                    nc.tensor.transpose(
                        tp[:, f * P:(f + 1) * P],
                        xn[:, a, f * P:(f + 1) * P], ident)
                nc.vector.tensor_copy(xt[:, :, a * P:(a + 1) * P], tp[:])
            return xt

        def proj_v(sp, xt, psum_pool, tag="pr"):
            for a in range(2):
                pv = psum_pool.tile([P, 2 * GW], F32, tag=tag, name="pv")
                for f in range(FCH):
                    nc.tensor.matmul(
                        pv[:], xt[:, f, a * P:(a + 1) * P], wv[:, f, :],
                        start=(f == 0), stop=(f == FCH - 1),
                    )
                s2 = sp * 2 + a
                nc.vector.tensor_add(
                    vht[:, s2, :, 0:DEPTH],
                    pv[:, 0:GW].rearrange("p (h d) -> p h d", h=HPC),
                    bvb[:].rearrange("p (h d) -> p h d", h=HPC),
                )

        def proj_qk(sp, xt, psum_pool, w01, w2d, b01, b2d, d01, d2f,
                    tag="pr"):
            sl = slice(sp * 2 * P, (sp + 1) * 2 * P)
            p01 = psum_pool.tile([P, 2 * P], F32, tag=tag, name="p01")
            for f in range(FCH):
                nc.tensor.matmul(
                    p01[:], w01[:, f, :], xt[:, f, :],
                    start=(f == 0), stop=(f == FCH - 1),
                )
            nc.vector.tensor_scalar_add(d01[:, sl], p01[:], b01[:])
            p2d = psum_pool.tile([P, 2 * P], F32, tag=tag, name="p2d")
            for f in range(FCH):
                nc.tensor.matmul(
                    p2d[:], w2d[:, f, :], xt[:, f, :],
                    start=(f == 0), stop=(f == FCH - 1),
                )
            nc.vector.tensor_scalar_add(d2f[:, sl], p2d[:], b2d[:])

        def normalize(outp, dst):
            rc = nrm_pool.tile([1, QB], F32, tag="rc", name="rc")
            nc.vector.reciprocal(rc[:], outp[DEPTH:DEPTH + 1, :])
            bc = nrm_pool.tile([DEPTH, QB], F32, tag="bc", name="bc")
            nc.gpsimd.partition_broadcast(bc[:], rc[:])
            nc.vector.tensor_mul(dst, outp[0:DEPTH, :], bc[:])

        def qk01(kc, qsl, lg_pool):
            lg = lg_pool.tile([P, 2, QB], F32, tag="lg", name="lg")
            nc.tensor.matmul(
                lg[:, 0, :], kT01[0:DEPTH, kc * P:(kc + 1) * P],
                qT01[0:DEPTH, qsl], start=True, stop=True)
            nc.tensor.matmul(
                lg[:, 1, :], kT01[DEPTH:P, kc * P:(kc + 1) * P],
                qT01[DEPTH:P, qsl], start=True, stop=True)
            return lg

        def av01(kc, lg, outp0, outp1):
            ext = ex_pool.tile([P, 2, QB], F32R, tag="ex", name="ex")
            nc.scalar.activation(ext[:], lg[:], AF.Exp, scale=SCALE)
            nc.tensor.matmul(
                outp0[:], vht[:, kc, 0, :], ext[:, 0, :],
                start=(kc == 0), stop=(kc == NKC - 1))
            nc.tensor.matmul(
                outp1[:], vht[:, kc, 1, :], ext[:, 1, :],
                start=(kc == 0), stop=(kc == NKC - 1))

        def qk2(j, qsl, lg_pool):
            lg = lg_pool.tile([P, 2, QB], F32, tag="lg", name="lg")
            nc.tensor.matmul(
                lg[:, 0, :], kT2f[0:DEPTH, (2 * j) * P:(2 * j + 1) * P],
                qT2f[0:DEPTH, qsl], start=True, stop=True)
            nc.tensor.matmul(
                lg[:, 1, :],
                kT2f[DEPTH:P, (2 * j + 1) * P:(2 * j + 2) * P],
                qT2f[DEPTH:P, qsl], start=True, stop=True)
            return lg

        def av2(j, lg, outp2):
            ext = ex_pool.tile([P, 2, QB], F32R, tag="ex", name="ex")
            nc.scalar.activation(ext[:], lg[:], AF.Exp, scale=SCALE)
            nc.tensor.matmul(
                outp2[:], vht[:, 2 * j, 2, :], ext[:, 0, :],
                start=(j == 0), stop=False)
            nc.tensor.matmul(
                outp2[:], vht[:, 2 * j + 1, 2, :], ext[:, 1, :],
                start=False, stop=(j == NKC // 2 - 1))

        # ---------- phase 1: project V then K (pipelined) ----------
        with (
            tc.tile_pool(name="tps", bufs=3, space="PSUM") as tps_pool,
            tc.tile_pool(name="pps", bufs=4, space="PSUM") as pps_pool,
        ):
            def vk_unit(kind, sp):
                xn = load_xn(xre_v if kind == "v" else xre_k, sp)
                return transpose_sp(xn, tps_pool, tag="tp")

            def vk_proj(kind, sp, xt):
                if kind == "v":
                    proj_v(sp, xt, pps_pool, tag="pp")
                else:
                    proj_qk(sp, xt, pps_pool, wk01, wk2d, bk01, bk2d,
                            kT01, kT2f, tag="pp")

            steps = [("v", sp) for sp in range(NSP)] +                     [("k", sp) for sp in range(NSP)]
            prev = None
            for kind, sp in steps:
                xt = vk_unit(kind, sp)
                if prev is not None:
                    vk_proj(prev[0], prev[1], prev[2])
                prev = (kind, sp, xt)
            vk_proj(prev[0], prev[1], prev[2])

        # ---------- phase 2: per q-block attention + output proj ----------
        with (
            tc.tile_pool(name="lg", bufs=2, space="PSUM") as lg_pool,
            tc.tile_pool(name="op", bufs=2, space="PSUM") as op_pool,
            tc.tile_pool(name="aux", bufs=2, space="PSUM") as aux_pool,
        ):
            def proj_q_main(sp):
                xn = load_xn(xre_q, sp)
                xt = transpose_sp(xn, aux_pool, tag="aux")
                proj_qk(sp, xt, aux_pool, wq01, wq2d, bq01, bq2d,
                        qT01, qT2f, tag="aux")

            def pass_h01(qb):
                qsl = slice(qb * QB, (qb + 1) * QB)
                o0 = op_pool.tile([DEPTH + 1, QB], F32, tag="outp",
                                  name="outp0")
                o1 = op_pool.tile([DEPTH + 1, QB], F32, tag="outp",
                                  name="outp1")
                prev = qk01(0, qsl, lg_pool)
                for kc in range(1, NKC):
                    cur = qk01(kc, qsl, lg_pool)
                    av01(kc - 1, prev, o0, o1)
                    prev = cur
                av01(NKC - 1, prev, o0, o1)
                normalize(o0, hout01[0:DEPTH, qsl])
                normalize(o1, hout01[DEPTH:P, qsl])

            def pass_h2(qb):
                qsl = slice(qb * QB, (qb + 1) * QB)
                o2 = op_pool.tile([DEPTH + 1, QB], F32, tag="outp",
                                  name="outp2")
                prev = qk2(0, qsl, lg_pool)
                for j in range(1, NKC // 2):
                    cur = qk2(j, qsl, lg_pool)
                    av2(j - 1, prev, o2)
                    prev = cur
                av2(NKC // 2 - 1, prev, o2)
                normalize(o2, hout2[:, qsl])

            def emit_c(qb):
                for m in range(4 * qb, 4 * qb + 4):
                    msl = slice(m * P, (m + 1) * P)
                    pa = aux_pool.tile([P, 512], F32, tag="aux", name="pa")
                    nc.tensor.matmul(pa[:], hout01[:, msl], wo0[:, 0:512],
                                     start=True, stop=False)
                    nc.tensor.matmul(pa[:], hout2[:, msl], wo1[:, 0:512],
                                     start=False, stop=True)
                    pb = aux_pool.tile([P, 256], F32, tag="aux", name="pb")
                    nc.tensor.matmul(pb[:], hout01[:, msl], wo0[:, 512:D],
                                     start=True, stop=False)
                    nc.tensor.matmul(pb[:], hout2[:, msl], wo1[:, 512:D],
                                     start=False, stop=True)
                    ot = out_pool.tile([P, D], F32, tag="ot", name="ot")
                    nc.vector.tensor_copy(ot[:, 0:512], pa[:])
                    nc.vector.tensor_copy(ot[:, 512:D], pb[:])
                    nc.sync.dma_start(OUT[msl, :], ot[:])

            for qb in range(NQB):
                proj_q_main(2 * qb)
                proj_q_main(2 * qb + 1)
                pass_h01(qb)
                if qb > 0:
                    emit_c(qb - 1)
                pass_h2(qb)
            emit_c(NQB - 1)


_NC = None


def build_nc(repeat=1, phases="ABC"):
    nc = bacc.Bacc("TRN2", target_bir_lowering=False, debug=False)
    XQ = nc.dram_tensor("xq", [S, D], F32, kind="ExternalInput").ap()
    XK = nc.dram_tensor("xk", [S, D], F32, kind="ExternalInput").ap()
    XV = nc.dram_tensor("xv", [S, D], F32, kind="ExternalInput").ap()
    WQ = nc.dram_tensor("wq", [D, GW], F32, kind="ExternalInput").ap()
    WK = nc.dram_tensor("wk", [D, GW], F32, kind="ExternalInput").ap()
    WV = nc.dram_tensor("wv", [D, GW], F32, kind="ExternalInput").ap()
    WO = nc.dram_tensor("wo", [GW, D], F32, kind="ExternalInput").ap()
    BQ = nc.dram_tensor("bq", [GW, 1], F32, kind="ExternalInput").ap()
    BK = nc.dram_tensor("bk", [GW, 1], F32, kind="ExternalInput").ap()
    BV = nc.dram_tensor("bv", [1, GW], F32, kind="ExternalInput").ap()
    OUT = nc.dram_tensor("out", [S, D], F32, kind="ExternalOutput").ap()
    tensors = (XQ, XK, XV, WQ, WK, WV, WO, BQ, BK, BV, OUT)
    from contextlib import ExitStack
    with tile.TileContext(nc) as tc:
        with ExitStack() as ctx:
            _emit(nc, tc, ctx, tensors, repeat=repeat, phases=phases)
    nc.compile()
    return nc


def _get_nc():
    global _NC
    if _NC is None:
        _NC = build_nc()
    return _NC


def kernel(**inputs):
    global LAST_RESULTS
    q = np.ascontiguousarray(np.asarray(inputs["q"], dtype=np.float32))
    k = np.ascontiguousarray(np.asarray(inputs["k"], dtype=np.float32))
    v = np.ascontiguousarray(np.asarray(inputs["v"], dtype=np.float32))
    Wq = np.asarray(inputs["Wq"], dtype=np.float32)
    Wk = np.asarray(inputs["Wk"], dtype=np.float32)
    Wv = np.asarray(inputs["Wv"], dtype=np.float32)
    Wo = np.asarray(inputs["Wo"], dtype=np.float32)
    bq = np.asarray(inputs["bq"], dtype=np.float32)
    bk = np.asarray(inputs["bk"], dtype=np.float32)
    bv = np.asarray(inputs["bv"], dtype=np.float32)
    bo = np.asarray(inputs["bo"], dtype=np.float32)
    # mask is all zeros by problem spec; ignored.

    nc = _get_nc()
    in_maps = []
    for c in range(N_CORES):
        b, g = c // 4, c % 4
        sl = slice(g * GW, (g + 1) * GW)
        in_maps.append({
            "xq": q[b], "xk": k[b], "xv": v[b],
            "wq": np.ascontiguousarray(Wq[:, sl]),
            "wk": np.ascontiguousarray(Wk[:, sl]),
            "wv": np.ascontiguousarray(Wv[:, sl]),
            "wo": np.ascontiguousarray(Wo[sl, :]),
            "bq": np.ascontiguousarray(bq[sl].reshape(GW, 1)),
            "bk": np.ascontiguousarray(bk[sl].reshape(GW, 1)),
            "bv": np.ascontiguousarray(bv[sl].reshape(1, GW)),
        })
    kwargs = {}
    if TRACE:
        kwargs = dict(trace=True)
    res = bass_utils.run_bass_kernel_spmd(nc, in_maps, list(range(N_CORES)),
                                          **kwargs)
    LAST_RESULTS = res
    out = np.zeros((B, S, D), dtype=np.float32)
    for c in range(N_CORES):
        out[c // 4] += res.results[c]["out"]
    out += bo
    return out
